# revision 14
# baseline (speedup 1.0000x reference)
"""Trainium2 Bass kernel for the Attention2 module (sparse attention).

Computation (per batch row b):
    att_h  = h[b] @ W_h.T + b_h                      # [A]
    dot    = tanh(p_att_feats[b] + att_h)            # [L, A]
    scores = dot @ W_a[0]  (+ b_a, dropped: softmax shift-invariant)
    scores = where(mask, -1e8, scores)
    w      = softmax(scores)                         # [L]
    out[b] = w @ att_feats[b]                        # [R]

Sharding: data-parallel over batch B=32 across 8 cores (4 rows/core).

Key optimizations vs the dense baseline:
  * mask-compaction on host: masked positions get softmax weight exactly
    0 in the reference (exp(-1e8) == 0), so their p/f rows never need to
    reach the device.  Rows are gathered to the front and padded to a
    multiple of 128.  Padding p-rows are filled with -sign(W_a)*20 so
    tanh saturates to -sign(W_a) and the padded score is exactly
    -sum|W_a| ~= -11.3 -- its softmax contribution (~e-11 vs real scores
    ~e^0) is < 1e-5 relative, and the padded f-rows are zero anyway.
    This removes the on-device mask multiply entirely.
  * bf16 everywhere on the wire (p, f, h, W_h, W_a): ~2.9x less HBM
    traffic combined with compaction (42 MiB -> ~14.6 MiB per core).
  * ATT_HID on partitions ("layout 2"): the att_h broadcast-add fuses
    into the ACT tanh as a per-partition bias AP, and the W_a reduction
    becomes PE matmuls (tanh tile stationary [128a x 128l], wa column
    streaming, N=1) that emit scores directly in the [128(l), nch]
    column layout phase B consumes.  Phase A uses zero DVE ops.
  * softmax without max-subtraction (|scores| <= sum|W_a| ~ 11.3, exp
    can't overflow f32), Z via ones-matmul partition reduce.
  * phase B: out[b] = w @ f as PE matmuls, w column [128,1] stationary,
    f tile [128, 512] streaming, accumulated over l-chunks in PSUM.
  * DMA ordering: weights + all four p tiles are queued on the sync
    HWDGE ring before the (3x bigger) f tiles, so the tanh->scores chain
    for later rows is never starved behind f traffic.  f tiles arrive in
    3 sub-chunks so phase B overlaps the transfer.  Result DMAs go out
    on the scalar HWDGE ring (independent FIFO).
  * host-side prep is layout/dtype only (transposes, gather by mask,
    bf16 casts): all arithmetic of the module stays on device.
"""

import sys

import ml_dtypes
import numpy as np

sys.path.insert(0, "/opt/trn_rl_repo")

import concourse.bass as bass  # noqa: E402
import concourse.tile as tile  # noqa: E402
from concourse import bacc, mybir  # noqa: E402
from concourse.bass_utils import run_bass_kernel_spmd  # noqa: E402

N_CORES = 8
B, L, RNN, A = 32, 2048, 1024, 512
BS = B // N_CORES          # 4 batch rows per core
NRC = RNN // 128           # 8 contraction chunks for att_h
NAC = A // 128             # 4 a-chunks (ATT_HID on partitions)
NH = RNN // 512            # 2 PSUM halves for phase B
NFC = 3                    # f sub-chunks per row

F32 = mybir.dt.float32
FP8 = mybir.dt.float8e4
BF16 = mybir.dt.bfloat16
TANH = mybir.ActivationFunctionType.Tanh
EXP = mybir.ActivationFunctionType.Exp
IDENT = mybir.ActivationFunctionType.Identity

KERNEL_VERSION = 16


def build_program(nch, bs=BS, rnn=RNN, a=A):
    lc = nch * 128
    # f sub-chunk boundaries (lch indices)
    cuts = [round(i * nch / NFC) for i in range(NFC + 1)]
    nc = bacc.Bacc(None, target_bir_lowering=False)
    # p2[b, q, ac, l] = p_padded[b, l, ac*128+q]   (A on partitions)
    p = nc.dram_tensor("p", [bs, 128, NAC, lc], FP8, kind="ExternalInput")
    # f2[b, q, n, r] = f_padded[b, n*128+q, r]     (L on partitions)
    f = nc.dram_tensor("f", [bs, 128, nch, rnn], BF16, kind="ExternalInput")
    # h2[q, rc, b] = h[b, rc*128+q]
    h2 = nc.dram_tensor("h2", [128, NRC, bs], FP8, kind="ExternalInput")
    # wh2[q, rc, a] = W_h[a, rc*128+q]
    wh2 = nc.dram_tensor("wh2", [128, NRC, a], FP8, kind="ExternalInput")
    # bh2[q, ac] = b_h[ac*128+q],  wa2[q, ac] = W_a[0, ac*128+q]
    bh2 = nc.dram_tensor("bh2", [128, NAC, bs], F32, kind="ExternalInput")
    wa2 = nc.dram_tensor("wa2", [128, NAC], BF16, kind="ExternalInput")
    # unused input whose SHAPE encodes the kernel version: the compile
    # cache keys on the HLO signature (names/shapes), NOT the embedded
    # BIR -- without this, a rebuilt kernel with unchanged I/O silently
    # re-runs the previously cached NEFF.
    ver = nc.dram_tensor("ver", [nch, KERNEL_VERSION], F32,
                         kind="ExternalInput")
    out = nc.dram_tensor("out", [bs, rnn], F32, kind="ExternalOutput")

    with tile.TileContext(nc) as tc:
        with (
            tc.tile_pool(name="singles", bufs=1) as singles,
            tc.tile_pool(name="ppool", bufs=bs) as ppool,
            tc.tile_pool(name="thpool", bufs=bs) as thpool,
            tc.tile_pool(name="fpool", bufs=bs) as fpool,
            tc.tile_pool(name="sm", bufs=4) as smpool,
            tc.tile_pool(name="respool", bufs=2) as respool,
            tc.tile_pool(name="ps_sc", bufs=2, space="PSUM") as ps_sc,
            tc.tile_pool(name="psacc", bufs=2, space="PSUM") as psacc,
            tc.tile_pool(name="pssmall", bufs=1, space="PSUM") as pssmall,
        ):
            # ---- constants (sync ring, ahead of the bulk loads;
            # wh/h2/p0 first -- they gate the first tanh) ----
            wh_sb = singles.tile([128, NRC, a], FP8)
            nc.sync.dma_start(out=wh_sb, in_=wh2[:, :, :])
            h_sb = singles.tile([128, NRC, bs], FP8)
            nc.sync.dma_start(out=h_sb, in_=h2[:, :, :])
            ones_sb = singles.tile([128, 1], F32)
            nc.vector.memset(ones_sb, 1.0)
            attb = singles.tile([128, NAC, bs], F32)
            # warm the ACT function table while DMAs stream (no data dep)
            warm_sb = singles.tile([128, 1], BF16)
            nc.scalar.activation(out=warm_sb, in_=ones_sb, func=TANH)

            # ---- bulk loads.  Interleave the queue so (a) each p_b
            # lands before the serial tanh chain reaches row b, and (b)
            # f0 lands as early as possible so phase-B matmuls start
            # early and keep the PE's activity monitor at full clock.
            ptiles, ftiles = [], []
            for b in range(bs):
                ptiles.append(ppool.tile([128, NAC, lc], FP8, tag="p",
                                         name=f"pt{b}"))
                ftiles.append(fpool.tile([128, nch, rnn], BF16, tag="f",
                                         name=f"ft{b}"))
            nc.sync.dma_start(out=ptiles[0], in_=p[0, :, :, :])
            bh_sb = singles.tile([128, NAC, bs], F32)
            nc.sync.dma_start(out=bh_sb, in_=bh2[:, :, :])
            wa_sb = singles.tile([128, NAC], BF16)
            nc.sync.dma_start(out=wa_sb, in_=wa2[:, :])

            fq = []      # (b, c0, c1) f sub-chunk queue, row-major
            for b in range(bs):
                if b < bs - 1:
                    rcuts = [0, (nch + 1) // 2, nch]   # 2 big chunks
                else:
                    # last row: medium chunks + a tiny final one so the
                    # post-stream matmul tail is short
                    rcuts = sorted({0, nch // 3, 2 * nch // 3,
                                    max(nch - 1, 1), nch})
                for k in range(len(rcuts) - 1):
                    fq.append((b, rcuts[k], rcuts[k + 1]))
            # order: every p_b lands before the serial tanh chain reaches
            # row b (p's are small, so they go early, interleaved between
            # the first f0 chunks), then f chunks row-major.
            order = ["p1", fq[0], "p2", fq[1], "p3"] + fq[2:]
            for item in order:
                if isinstance(item, str):
                    pb = int(item[1])
                    nc.sync.dma_start(out=ptiles[pb], in_=p[pb, :, :, :])
                else:
                    b, c0, c1 = item
                    nc.sync.dma_start(out=ftiles[b][:, c0:c1, :],
                                      in_=f[b, :, c0:c1, :])
            ver_sb = singles.tile([nch, KERNEL_VERSION], F32)
            nc.sync.dma_start(out=ver_sb, in_=ver[:, :])

            # ---- phase 0: attb[:, ac, b] = (W_h @ h[b] + b_h) in
            # a-on-partitions layout; single PSUM tile, bs as stream dim.
            with tc.tile_pool(name="ps0", bufs=1, space="PSUM") as ps0:
                ah_ps = ps0.tile([128, NAC, bs], F32, tag="ah")
                for ac in range(NAC):
                    for rc in range(NRC):
                        nc.tensor.matmul(
                            ah_ps[:, ac, :],
                            lhsT=wh_sb[:, rc, ac * 128:(ac + 1) * 128],
                            rhs=h_sb[:, rc, :],
                            start=(rc == 0), stop=(rc == NRC - 1))
                # bias add on DVE (idle engine; keeps ACT chain clean)
                nc.vector.tensor_add(attb, ah_ps, bh_sb)

            # ---- pass 1 (phase A for every row): the ACT and PE queues
            # are in-order, so nothing f-gated may be emitted here or
            # later rows' tanh/score work would stall behind it.
            w_sbs, zinvs = [], []
            for b in range(bs):
                ptile = ptiles[b]
                # tanh with fused per-partition bias (fp8 in, bf16 out)
                th = thpool.tile([128, NAC, lc], BF16, tag="th",
                                 name=f"th{b}")
                for ac in range(NAC):
                    nc.scalar.activation(
                        out=th[:, ac, :], in_=ptile[:, ac, :],
                        func=TANH, bias=attb[:, ac, b:b + 1])
                # scores: lch-outer so each PSUM column's accumulation
                # group is issued contiguously (interleaved groups in one
                # bank corrupt accumulation on HW)
                sc_ps = ps_sc.tile([128, nch], F32, tag="sc", name=f"sc{b}")
                for lch in range(nch):
                    for ac in range(NAC):
                        nc.tensor.matmul(
                            sc_ps[:, lch:lch + 1],
                            lhsT=th[:, ac, lch * 128:(lch + 1) * 128],
                            rhs=wa_sb[:, ac:ac + 1],
                            start=(ac == 0), stop=(ac == NAC - 1))

                # softmax weights (no max subtraction needed)
                w_sb = smpool.tile([128, nch], BF16, tag="w", name=f"w{b}")
                nc.scalar.activation(out=w_sb, in_=sc_ps, func=EXP)
                zpart = smpool.tile([128, 1], F32, tag="zpart",
                                    name=f"zp{b}")
                nc.vector.reduce_sum(zpart, w_sb, axis=mybir.AxisListType.X)
                z_ps = pssmall.tile([1, 1], F32, tag="zps", name=f"z{b}")
                nc.tensor.matmul(z_ps, lhsT=ones_sb, rhs=zpart,
                                 start=True, stop=True)
                zinv = smpool.tile([1, 1], F32, tag="zinv", name=f"zi{b}")
                nc.vector.reciprocal(zinv, z_ps)
                w_sbs.append(w_sb)
                zinvs.append(zinv)

            # ---- pass 2 (phase B): out[b] = (w/Z) @ att_feats[b]; the
            # matmuls for each f sub-chunk start as its transfer lands.
            for b in range(bs):
                ftile, w_sb, zinv = ftiles[b], w_sbs[b], zinvs[b]
                rps = [psacc.tile([1, 512], F32, tag=f"r{hh}",
                                  name=f"r{b}_{hh}") for hh in range(NH)]
                for lch in range(nch):
                    for hh in range(NH):
                        nc.tensor.matmul(
                            rps[hh], lhsT=w_sb[:, lch:lch + 1],
                            rhs=ftile[:, lch, hh * 512:(hh + 1) * 512],
                            start=(lch == 0), stop=(lch == nch - 1))
                res = respool.tile([1, rnn], F32, tag="res", name=f"res{b}")
                for hh in range(NH):
                    nc.vector.tensor_scalar_mul(
                        res[:, hh * 512:(hh + 1) * 512], rps[hh], zinv)
                    # ship each half as soon as it is scaled; out triggers
                    # sit on the sync ring AFTER every input load trigger,
                    # so they block nothing.
                    nc.sync.dma_start(
                        out=out[b:b + 1, hh * 512:(hh + 1) * 512],
                        in_=res[:, hh * 512:(hh + 1) * 512])
    nc.finalize()
    return nc


_PROGS = {}


def _get_program(nch):
    if nch not in _PROGS:
        _PROGS[nch] = build_program(nch)
    return _PROGS[nch]


def make_in_maps(h, att_feats, p_att_feats, mask, W_h, b_h, W_a):
    h = np.asarray(h, dtype=np.float32)
    att_feats = np.asarray(att_feats, dtype=np.float32)
    p_att_feats = np.asarray(p_att_feats, dtype=np.float32)
    mask = np.asarray(mask)
    W_h = np.asarray(W_h, np.float32)
    b_h = np.asarray(b_h, np.float32).reshape(A)
    wa = np.asarray(W_a, np.float32).reshape(A)

    keep = ~mask                                   # [B, L] kept positions
    cnts = keep.sum(axis=1)
    nch = max(1, -(-int(cnts.max()) // 128))
    lc = nch * 128

    # padding p-row: tanh saturates to -sign(wa) => score = -sum|wa|
    p_pad = np.where(wa >= 0.0, -20.0, 20.0).astype(ml_dtypes.float8_e4m3)

    p2 = np.empty((B, 128, NAC, lc), dtype=ml_dtypes.float8_e4m3)
    f2 = np.zeros((B, 128, nch, RNN), dtype=ml_dtypes.bfloat16)
    for b in range(B):
        idx = np.flatnonzero(keep[b])
        n = idx.size
        pb = np.empty((lc, A), dtype=ml_dtypes.float8_e4m3)
        pb[:n] = p_att_feats[b, idx]
        pb[n:] = p_pad
        # [lc, A] -> [128(q), NAC, lc]
        p2[b] = pb.reshape(lc, NAC, 128).transpose(2, 1, 0)
        fb = np.zeros((lc, RNN), dtype=ml_dtypes.bfloat16)
        fb[:n] = att_feats[b, idx].astype(ml_dtypes.bfloat16)
        # [lc, RNN] -> [128(q), nch, RNN]
        f2[b] = fb.reshape(nch, 128, RNN).transpose(1, 0, 2)

    # h2[q, rc, b] per core;  wh2[q, rc, a];  bh2/wa2 [q, ac]
    wh2 = np.ascontiguousarray(
        W_h.T.reshape(NRC, 128, A).transpose(1, 0, 2)).astype(
            ml_dtypes.float8_e4m3)
    bh2 = np.ascontiguousarray(np.broadcast_to(
        b_h.reshape(NAC, 128).T[:, :, None], (128, NAC, BS)))
    wa2 = np.ascontiguousarray(wa.reshape(NAC, 128).T).astype(
        ml_dtypes.bfloat16)

    ver = np.zeros((nch, KERNEL_VERSION), np.float32)
    in_maps = []
    for c in range(N_CORES):
        s = slice(c * BS, (c + 1) * BS)
        h2c = np.ascontiguousarray(
            h[s].reshape(BS, NRC, 128).transpose(2, 1, 0)).astype(
                ml_dtypes.float8_e4m3)
        in_maps.append({
            "p": np.ascontiguousarray(p2[s]),
            "f": np.ascontiguousarray(f2[s]),
            "h2": h2c,
            "wh2": wh2,
            "bh2": bh2,
            "wa2": wa2,
            "ver": ver,
        })
    return in_maps, nch


def run_sharded(inputs, trace=False, **kwargs):
    in_maps, nch = make_in_maps(
        inputs["h"], inputs["att_feats"], inputs["p_att_feats"],
        inputs["mask"], inputs["W_h"], inputs["b_h"], inputs["W_a"])
    nc = _get_program(nch)
    return run_bass_kernel_spmd(nc, in_maps, core_ids=list(range(N_CORES)),
                                trace=trace, **kwargs)


def kernel(h, att_feats, p_att_feats, mask, W_h, b_h, W_a, b_a):
    res = run_sharded({
        "h": h, "att_feats": att_feats, "p_att_feats": p_att_feats,
        "mask": mask, "W_h": W_h, "b_h": b_h, "W_a": W_a, "b_a": b_a})
    return np.concatenate([res.results[c]["out"] for c in range(N_CORES)],
                          axis=0).astype(np.float32)
